# revision 12
# baseline (speedup 1.0000x reference)
"""Trainium2 Bass kernel for nn_CP_L3_sparse_outer (v4, bf16).

Math (per batch row b):
    s2[b] = sum_d U2[d] * z[b, d]
    s3[b] = sum_d U3[d] * z[b, d]
    out[b, o] = (s2[b] * s3[b]) * sum_d (U1[d] * z[b, d]) * W[o, d] + bias[o]

Sharding: data-parallel over batch B=8192 across 8 NeuronCores
(B_loc = 1024 rows per core); W / U1 / U2 / U3 / bias replicated.

All-bf16 pipeline (measured rel-err 0.29% vs the 2e-2 gate). PE runs ONLY
the z transposes and the main matmul stream; everything else lives on
DVE/ACT/DMA so the tensor engine never starves:

  - Constants arrive pre-broadcast from the host (identity, U2/U3/bias
    replicated across partitions, U1 pre-tiled [128, 32]) on the sync
    HWDGE queue; z row-tiles also load via sync (the gpsimd SWDGE queue
    is reserved for W-slab + output traffic so neither blocks the other).
  - Per batch tile bt: transpose 32 chunks (bf16 = 1 cyc/row) in 4-chunk
    groups through PSUM; ACT copies into resident ztbig = z.T
    [128 d, k * 1024 + b]. s2/s3 on DVE: scalar_tensor_tensor
    (znat * u2b) with accum_out -> s2col [128, 1] per tile; c = s2*s3
    lands directly in ccol[:, bt] (per-partition scalar for eviction).
    Then U1 folds into zt in place (DVE, per-partition, per chunk) --
    the only gate for the main matmuls of this tile.
  - Main matmul, output-natural: per o-chunk (8 x 512): wt slab
    [128 d, 32 k, 512 o] streamed in two k-halves (first chunk's halves
    hoisted before phase A so oc0 can start ~10us in); per bt:
    psum[128 b, 512 o] += zt[k, bt] (stationary) @ wt[k, oc] (moving),
    evicted with ONE DVE op: (psum * ccol) + biasb; batched out DMA per
    oc (split for the last chunk to shorten the drain tail).

Host prep is dtype/layout only: bf16 casts, W.T contiguous, constant
replication across the 128 partitions (np.broadcast_to), zero FLOPs.
"""

import os
import sys

import numpy as np

if "/opt/trn_rl_repo" not in sys.path:
    sys.path.insert(0, "/opt/trn_rl_repo")

import concourse.bass as bass
from concourse import bacc
import concourse.mybir as mybir
import concourse.tile as tile

P = 128
D = 4096
O = 4096
B = 8192
NCORES = 8
BLOC = B // NCORES          # 1024 batch rows per core
KC = D // P                 # 32 contraction chunks
BT = BLOC // P              # 8 batch tiles of 128
OC = O // 512               # 8 output chunks of 512
KH = KC // 2                # k-half per W slab DMA
F32 = mybir.dt.float32
BF16 = mybir.dt.bfloat16
MULT = mybir.AluOpType.mult
ADD = mybir.AluOpType.add
COPY = mybir.ActivationFunctionType.Copy


def build_nc() -> bass.Bass:
    nc = bacc.Bacc(trn_type="TRN2")

    z_d = nc.dram_tensor("z", [BLOC, D], BF16, kind="ExternalInput")
    wt_d = nc.dram_tensor("wt", [D, O], BF16, kind="ExternalInput")
    u1_d = nc.dram_tensor("u1", [P, KC], F32, kind="ExternalInput")
    u2b_d = nc.dram_tensor("u2b", [P, D], BF16, kind="ExternalInput")
    u3b_d = nc.dram_tensor("u3b", [P, D], BF16, kind="ExternalInput")
    biasb_d = nc.dram_tensor("biasb", [P, O], BF16, kind="ExternalInput")
    ident_d = nc.dram_tensor("ident", [P, P], BF16, kind="ExternalInput")
    out_d = nc.dram_tensor("out", [BLOC, O], F32, kind="ExternalOutput")

    zview = z_d[:].rearrange("(t p) d -> p t d", p=P)          # [128, 8, 4096]
    wview = wt_d[:].rearrange("(k p) o -> p k o", p=P)         # [128, 32, 4096]
    oview = out_d[:].rearrange("(t p) o -> p t o", p=P)        # [128, 8, 4096]

    with tile.TileContext(nc) as tc:
        with (
            tc.tile_pool(name="const", bufs=1) as const,
            tc.tile_pool(name="ztp", bufs=1) as ztp,
            tc.tile_pool(name="znat", bufs=2) as znatp,
            tc.tile_pool(name="wslab", bufs=2) as wslabp,
            tc.tile_pool(name="onat", bufs=2) as onatp,
            tc.tile_pool(name="pmain", bufs=4, space="PSUM") as pmain,
            tc.tile_pool(name="ptr", bufs=3, space="PSUM") as ptr,
        ):
            # ---- constants: all pre-broadcast on host, sync HWDGE ----
            identity_b = const.tile([P, P], BF16)
            nc.sync.dma_start(identity_b[:], ident_d[:])
            u1sb = const.tile([P, KC], F32)
            nc.sync.dma_start(u1sb[:], u1_d[:])
            u2b = const.tile([P, D], BF16)
            nc.sync.dma_start(u2b[:], u2b_d[:])
            u3b = const.tile([P, D], BF16)
            nc.sync.dma_start(u3b[:], u3b_d[:])
            biasb = const.tile([P, O], BF16)
            nc.sync.dma_start(biasb[:], biasb_d[:])
            ccol = const.tile([P, BT], F32)
            s2col = const.tile([P, 1], F32)
            s3col = const.tile([P, 1], F32)

            # warm-up transpose: first PE instruction
            ptw = ptr.tile([P, 512], BF16, name="pt", tag="pt")
            nc.tensor.transpose(ptw[:, 0:P], identity_b[:], identity_b[:])

            # zT resident: [128 d_in, k * BLOC + b]
            ztbig = ztp.tile([P, KC * BLOC], BF16)
            zt3 = ztbig[:].rearrange("p (k r) -> p k r", r=BLOC)

            # hoist the first W slab (two k-halves) ahead of phase A
            ws0 = wslabp.tile([P, KC, 512], BF16, name="wslab")
            for h in range(2):
                nc.gpsimd.dma_start(
                    ws0[:, h * KH : (h + 1) * KH, :],
                    wview[:, h * KH : (h + 1) * KH, 0:512],
                )

            # ---- phase A (+ DVE s2/s3/c/U1) per batch tile ----
            for bt in range(BT):
                znat = znatp.tile([P, D], BF16, name="znat")
                nc.sync.dma_start(znat[:], zview[:, bt, :])
                for g in range(KC // 4):
                    pt = ptr.tile([P, 512], BF16, name="pt", tag="pt")
                    for i in range(4):
                        nc.tensor.transpose(
                            pt[:, i * P : (i + 1) * P],
                            znat[:, (g * 4 + i) * P : (g * 4 + i + 1) * P],
                            identity_b[:],
                        )
                    nc.scalar.activation(
                        zt3[:, g * 4 : g * 4 + 4, bt * P : (bt + 1) * P],
                        pt[:].rearrange("p (k r) -> p k r", r=P),
                        COPY,
                    )
                # s2/s3 for this tile on DVE (free-dim accumulate); the
                # scratch outputs live in the (still idle) onat pool
                sscr = onatp.tile([P, D], BF16, name="onat")
                nc.vector.scalar_tensor_tensor(
                    sscr[:], znat[:], 1.0, u2b[:], MULT, MULT,
                    accum_out=s2col[:],
                )
                sscr = onatp.tile([P, D], BF16, name="onat")
                nc.vector.scalar_tensor_tensor(
                    sscr[:], znat[:], 1.0, u3b[:], MULT, MULT,
                    accum_out=s3col[:],
                )
                nc.vector.tensor_mul(ccol[:, bt : bt + 1], s2col[:], s3col[:])
                # fold U1 into zt in place (per-partition scalar per chunk)
                for k in range(KC):
                    nc.vector.tensor_scalar_mul(
                        zt3[:, k, bt * P : (bt + 1) * P],
                        zt3[:, k, bt * P : (bt + 1) * P],
                        u1sb[:, k : k + 1],
                    )

            # ---- main matmul, output-natural psum [b, o] ----
            for oc in range(OC):
                if oc == 0:
                    ws = ws0
                else:
                    ws = wslabp.tile([P, KC, 512], BF16, name="wslab")
                    for h in range(2):
                        nc.gpsimd.dma_start(
                            ws[:, h * KH : (h + 1) * KH, :],
                            wview[:, h * KH : (h + 1) * KH,
                                  oc * 512 : (oc + 1) * 512],
                        )
                onat = onatp.tile([P, BT, 512], F32, name="onat")
                for bt in range(BT):
                    pm = pmain.tile([P, 512], F32, name="pm", tag="pmain")
                    for k in range(KC):
                        nc.tensor.matmul(
                            pm[:],
                            zt3[:, k, bt * P : (bt + 1) * P],
                            ws[:, k, :],
                            start=(k == 0),
                            stop=(k == KC - 1),
                        )
                    nc.vector.scalar_tensor_tensor(
                        onat[:, bt, :],
                        pm[:],
                        ccol[:, bt : bt + 1],
                        biasb[:, oc * 512 : (oc + 1) * 512],
                        MULT,
                        ADD,
                    )
                if oc == OC - 1:
                    # split the last store so the drain tail is half as long
                    nc.gpsimd.dma_start(
                        oview[:, 0 : BT // 2, oc * 512 : (oc + 1) * 512],
                        onat[:, 0 : BT // 2, :],
                    )
                    nc.gpsimd.dma_start(
                        oview[:, BT // 2 : BT, oc * 512 : (oc + 1) * 512],
                        onat[:, BT // 2 : BT, :],
                    )
                else:
                    nc.gpsimd.dma_start(
                        oview[:, :, oc * 512 : (oc + 1) * 512], onat[:]
                    )

    nc.finalize()
    return nc


_NC_CACHE = {}


def get_nc() -> bass.Bass:
    if "nc" not in _NC_CACHE:
        _NC_CACHE["nc"] = build_nc()
    return _NC_CACHE["nc"]


def kernel(z, U1, U2, U3, W, b):
    import ml_dtypes
    from concourse.bass_utils import run_bass_kernel_spmd

    bf = ml_dtypes.bfloat16
    z = np.ascontiguousarray(np.asarray(z, dtype=np.float32)).reshape(B, D)
    zq = z.astype(bf)
    wt = np.ascontiguousarray(np.asarray(W, dtype=np.float32).T).astype(bf)
    u1t = np.ascontiguousarray(
        np.asarray(U1, dtype=np.float32).reshape(KC, P).T
    )
    u2q = np.asarray(U2, dtype=np.float32).astype(bf)
    u3q = np.asarray(U3, dtype=np.float32).astype(bf)
    u2b = np.ascontiguousarray(np.broadcast_to(u2q, (P, D)))
    u3b = np.ascontiguousarray(np.broadcast_to(u3q, (P, D)))
    biasq = np.asarray(b, dtype=np.float32).astype(bf)
    biasb = np.ascontiguousarray(np.broadcast_to(biasq, (P, O)))
    ident = np.eye(P, dtype=bf)

    nc = get_nc()
    in_maps = [
        {
            "z": zq[c * BLOC : (c + 1) * BLOC],
            "wt": wt,
            "u1": u1t,
            "u2b": u2b,
            "u3b": u3b,
            "biasb": biasb,
            "ident": ident,
        }
        for c in range(NCORES)
    ]
    res = run_bass_kernel_spmd(
        nc,
        in_maps,
        core_ids=list(range(NCORES)),
        trace=bool(int(os.environ.get("KERNEL_TRACE", "0"))),
    )
    if res.exec_time_ns is not None:
        print(f"HW exec time: {res.exec_time_ns} ns", file=sys.stderr)
    kernel.last_results = res
    return np.concatenate([res.results[c]["out"] for c in range(NCORES)], axis=0)


# revision 13
# speedup vs baseline: 1.2075x; 1.2075x over previous
"""Trainium2 Bass kernel for nn_CP_L3_sparse_outer (v5, bf16).

Math (per batch row b):
    s2[b] = sum_d U2[d] * z[b, d]
    s3[b] = sum_d U3[d] * z[b, d]
    out[b, o] = (s2[b] * s3[b]) * sum_d (U1[d] * z[b, d]) * W[o, d] + bias[o]

Sharding: data-parallel over batch B=8192 across 8 NeuronCores
(B_loc = 1024 rows per core); W / U1 / U2 / U3 / bias replicated.

All-bf16 pipeline (measured rel-err 0.29% vs the 2e-2 gate), main matmul
output-natural (psum [b, o]) so there are no output transposes. The
schedule is built for overlap: everything is emitted per batch-tile-PAIR
so the main matmul stream can start ~12us in and fill the z-DMA gaps
(v3's phase-serial variant left 37us of PE idle; a variant with s2/s3 on
DVE ran the whole PE at 2.0 GHz -- P0 power state -- so s2/s3 stays on
PE and DVE only does the U1 fold and psum evictions).

  A. z bf16 row-tiles via SWDGE; PE transposes (bf16 = 1 cyc/row) in
     4-chunk groups through PSUM; ACT copies into resident
     ztbig = z.T [128 d, k(32) * 1024 b], raw.
  B. Per PAIR of batch tiles: s2/s3 on PE: psum[64, 256] += u23pad.T @
     ztRAW over 32 k (U2 -> stationary col 0, U3 -> col 32 so evictions
     read 32-aligned psum partitions).
  D. U1 folds into zt in place per (k, pair) on DVE (u1 on partitions)
     -- the only gate for that pair's main matmuls.
  C. After the last pair: c = s2*s3 (DVE) -> 8 one-column micro-matmuls
     -> ccol [128 b, 8 bt] (c becomes a per-partition scalar at
     eviction); bias broadcast via ones-outer-product matmuls -> biasb.
  E. Per o-chunk (8 x 512): wt slab [128 d, 32 k, 512 o] via SWDGE (the
     first slab is split in two k-halves and hoisted between z0 and z1
     so oc0 can start as soon as one batch pair is ready); per bt:
     psum[128 b, 512 o] += zt[k, bt] (stationary) @ wt[k, oc] (moving);
     evict with ONE DVE op: (psum * ccol) + biasb; batched out DMA per
     oc, split in half for the last chunk to shorten the drain tail.

Host prep is dtype/layout only: bf16 casts, W.T contiguous, u1/u23
pre-tiled to [128, 32(,2)] (partition-contiguous one-shot loads), and an
identity matrix (replaces on-device iota codegen that delayed the z DMA
queue).
"""

import os
import sys

import numpy as np

if "/opt/trn_rl_repo" not in sys.path:
    sys.path.insert(0, "/opt/trn_rl_repo")

import concourse.bass as bass
from concourse import bacc
import concourse.mybir as mybir
import concourse.tile as tile

P = 128
D = 4096
O = 4096
B = 8192
NCORES = 8
BLOC = B // NCORES          # 1024 batch rows per core
KC = D // P                 # 32 contraction chunks
BT = BLOC // P              # 8 batch tiles of 128
NP = BT // 2                # 4 batch-tile pairs
OC = O // 512               # 8 output chunks of 512
KH = KC // 2                # k-half for the hoisted first W slab
F32 = mybir.dt.float32
BF16 = mybir.dt.bfloat16
MULT = mybir.AluOpType.mult
ADD = mybir.AluOpType.add
COPY = mybir.ActivationFunctionType.Copy


def build_nc() -> bass.Bass:
    nc = bacc.Bacc(trn_type="TRN2")

    z_d = nc.dram_tensor("z", [BLOC, D], BF16, kind="ExternalInput")
    wt_d = nc.dram_tensor("wt", [D, O], BF16, kind="ExternalInput")
    u1_d = nc.dram_tensor("u1", [P, KC], F32, kind="ExternalInput")
    u23_d = nc.dram_tensor("u23", [P, KC, 2], BF16, kind="ExternalInput")
    bias_d = nc.dram_tensor("bias", [O], BF16, kind="ExternalInput")
    ident_d = nc.dram_tensor("ident", [P, P], BF16, kind="ExternalInput")
    out_d = nc.dram_tensor("out", [BLOC, O], F32, kind="ExternalOutput")

    zview = z_d[:].rearrange("(t p) d -> p t d", p=P)          # [128, 8, 4096]
    wview = wt_d[:].rearrange("(k p) o -> p k o", p=P)         # [128, 32, 4096]
    oview = out_d[:].rearrange("(t p) o -> p t o", p=P)        # [128, 8, 4096]

    with tile.TileContext(nc) as tc:
        with (
            tc.tile_pool(name="const", bufs=1) as const,
            tc.tile_pool(name="ztp", bufs=1) as ztp,
            tc.tile_pool(name="znat", bufs=2) as znatp,
            tc.tile_pool(name="wslab", bufs=2) as wslabp,
            tc.tile_pool(name="onat", bufs=2) as onatp,
            tc.tile_pool(name="pmain", bufs=4, space="PSUM") as pmain,
            tc.tile_pool(name="ptr", bufs=2, space="PSUM") as ptr,
            tc.tile_pool(name="pmisc", bufs=2, space="PSUM") as pmisc,
        ):
            # ---- constants (host-tiled, partition-contiguous loads) ----
            identity_b = const.tile([P, P], BF16)
            nc.sync.dma_start(identity_b[:], ident_d[:])
            ones1 = const.tile([1, P], BF16)
            nc.vector.memset(ones1[:], 1.0)
            onef = const.tile([1, 1], F32)
            nc.vector.memset(onef[:], 1.0)
            u1sb = const.tile([P, KC], F32)
            nc.sync.dma_start(u1sb[:], u1_d[:])
            u23sb = const.tile([P, KC, 2], BF16)
            nc.sync.dma_start(u23sb[:], u23_d[:])
            # s2/s3 psum rows must land on 32-aligned partitions: put U2 in
            # stationary column 0 and U3 in column 32 of a zero-padded lhsT.
            u23pad = const.tile([P, KC, 64], BF16)
            nc.vector.memset(u23pad[:], 0.0)
            nc.vector.tensor_copy(u23pad[:, :, 0:1], u23sb[:, :, 0:1])
            nc.vector.tensor_copy(u23pad[:, :, 32:33], u23sb[:, :, 1:2])
            biasrow = znatp.tile([1, O], BF16, name="znat")
            nc.sync.dma_start(biasrow[:], bias_d[:].rearrange("(a o) -> a o", a=1))
            biasb = const.tile([P, O], BF16)
            t2row = const.tile([1, BLOC], F32)
            t3row = const.tile([1, BLOC], F32)
            ccol = const.tile([P, BT], F32)

            # warm-up transpose: first PE instruction, one small DMA dep
            ptw = ptr.tile([P, 512], BF16, name="pt", tag="pt")
            nc.tensor.transpose(ptw[:, 0:P], identity_b[:], identity_b[:])

            # zT resident: [128 d_in, k * BLOC + b]
            ztbig = ztp.tile([P, KC * BLOC], BF16)
            zt3 = ztbig[:].rearrange("p (k r) -> p k r", r=BLOC)

            ws0 = None
            # ---- phases A/B/D per batch-tile pair ----
            for pr in range(NP):
                for half in range(2):
                    bt = pr * 2 + half
                    znat = znatp.tile([P, D], BF16, name="znat")
                    nc.gpsimd.dma_start(znat[:], zview[:, bt, :])
                    if bt == 0:
                        # hoist the first W slab (two k-halves) right after
                        # z0 on the SWDGE queue so oc0 can start early
                        ws0 = wslabp.tile([P, KC, 512], BF16, name="wslab")
                        for h in range(2):
                            nc.gpsimd.dma_start(
                                ws0[:, h * KH : (h + 1) * KH, :],
                                wview[:, h * KH : (h + 1) * KH, 0:512],
                            )
                    for g in range(KC // 4):
                        pt = ptr.tile([P, 512], BF16, name="pt", tag="pt")
                        for i in range(4):
                            nc.tensor.transpose(
                                pt[:, i * P : (i + 1) * P],
                                znat[:, (g * 4 + i) * P : (g * 4 + i + 1) * P],
                                identity_b[:],
                            )
                        nc.scalar.activation(
                            zt3[:, g * 4 : g * 4 + 4, bt * P : (bt + 1) * P],
                            pt[:].rearrange("p (k r) -> p k r", r=P),
                            COPY,
                        )
                # B: s2/s3 for this pair from RAW zt
                sl = slice(pr * 256, (pr + 1) * 256)
                ps23 = pmisc.tile([64, 256], F32, name="ps23", tag="pmisc")
                for k in range(KC):
                    nc.tensor.matmul(
                        ps23[:],
                        u23pad[:, k, :],
                        zt3[:, k, sl],
                        start=(k == 0),
                        stop=(k == KC - 1),
                    )
                nc.vector.tensor_copy(t2row[0:1, sl], ps23[0:1, :])
                nc.vector.tensor_copy(t3row[0:1, sl], ps23[32:33, :])
                # D: fold U1 into this pair's zt in place
                for k in range(KC):
                    nc.vector.tensor_scalar_mul(
                        zt3[:, k, sl], zt3[:, k, sl], u1sb[:, k : k + 1]
                    )

            # ---- phase C: c = s2*s3 (in place) -> ccol [128 b, bt] ----
            nc.vector.tensor_mul(t2row[0:1, :], t2row[0:1, :], t3row[0:1, :])
            pc = pmisc.tile([P, BT], F32, name="pc", tag="pmisc")
            for g in range(BT):
                nc.tensor.matmul(
                    pc[:, g : g + 1],
                    t2row[0:1, g * P : (g + 1) * P],
                    onef[0:1, 0:1],
                    start=True, stop=True,
                )
            nc.vector.tensor_copy(ccol[:], pc[:])

            # bias broadcast across partitions: biasb[p, o] = bias[o]
            for oc in range(OC):
                pb = pmisc.tile([P, 512], F32, name="pb", tag="pmisc")
                nc.tensor.matmul(
                    pb[:], ones1[:], biasrow[0:1, oc * 512 : (oc + 1) * 512],
                    start=True, stop=True,
                )
                nc.scalar.activation(biasb[:, oc * 512 : (oc + 1) * 512], pb[:], COPY)

            # ---- phase E: main matmul, output-natural psum [b, o] ----
            for oc in range(OC):
                if oc == 0:
                    ws = ws0
                else:
                    ws = wslabp.tile([P, KC, 512], BF16, name="wslab")
                    nc.gpsimd.dma_start(
                        ws[:], wview[:, :, oc * 512 : (oc + 1) * 512]
                    )
                onat = onatp.tile([P, BT, 512], F32, name="onat")
                for bt in range(BT):
                    pm = pmain.tile([P, 512], F32, name="pm", tag="pmain")
                    for k in range(KC):
                        nc.tensor.matmul(
                            pm[:],
                            zt3[:, k, bt * P : (bt + 1) * P],
                            ws[:, k, :],
                            start=(k == 0),
                            stop=(k == KC - 1),
                        )
                    nc.vector.scalar_tensor_tensor(
                        onat[:, bt, :],
                        pm[:],
                        ccol[:, bt : bt + 1],
                        biasb[:, oc * 512 : (oc + 1) * 512],
                        MULT,
                        ADD,
                    )
                if oc == OC - 1:
                    # split the last store so the drain tail is half as long
                    nc.gpsimd.dma_start(
                        oview[:, 0 : BT // 2, oc * 512 : (oc + 1) * 512],
                        onat[:, 0 : BT // 2, :],
                    )
                    nc.gpsimd.dma_start(
                        oview[:, BT // 2 : BT, oc * 512 : (oc + 1) * 512],
                        onat[:, BT // 2 : BT, :],
                    )
                else:
                    nc.gpsimd.dma_start(
                        oview[:, :, oc * 512 : (oc + 1) * 512], onat[:]
                    )

    nc.finalize()
    return nc


_NC_CACHE = {}


def get_nc() -> bass.Bass:
    if "nc" not in _NC_CACHE:
        _NC_CACHE["nc"] = build_nc()
    return _NC_CACHE["nc"]


def kernel(z, U1, U2, U3, W, b):
    import ml_dtypes
    from concourse.bass_utils import run_bass_kernel_spmd

    bf = ml_dtypes.bfloat16
    z = np.ascontiguousarray(np.asarray(z, dtype=np.float32)).reshape(B, D)
    zq = z.astype(bf)
    wt = np.ascontiguousarray(np.asarray(W, dtype=np.float32).T).astype(bf)
    u1t = np.ascontiguousarray(
        np.asarray(U1, dtype=np.float32).reshape(KC, P).T
    )
    u23 = np.stack(
        [np.asarray(U2, dtype=np.float32), np.asarray(U3, dtype=np.float32)], 1
    )
    u23t = np.ascontiguousarray(
        u23.reshape(KC, P, 2).transpose(1, 0, 2)
    ).astype(bf)
    bias = np.asarray(b, dtype=np.float32).astype(bf)
    ident = np.eye(P, dtype=bf)

    nc = get_nc()
    in_maps = [
        {
            "z": zq[c * BLOC : (c + 1) * BLOC],
            "wt": wt,
            "u1": u1t,
            "u23": u23t,
            "bias": bias,
            "ident": ident,
        }
        for c in range(NCORES)
    ]
    res = run_bass_kernel_spmd(
        nc,
        in_maps,
        core_ids=list(range(NCORES)),
        trace=bool(int(os.environ.get("KERNEL_TRACE", "0"))),
    )
    if res.exec_time_ns is not None:
        print(f"HW exec time: {res.exec_time_ns} ns", file=sys.stderr)
    kernel.last_results = res
    return np.concatenate([res.results[c]["out"] for c in range(NCORES)], axis=0)


# revision 17
# speedup vs baseline: 1.3043x; 1.0802x over previous
"""Trainium2 Bass kernel for nn_CP_L3_sparse_outer (v5, bf16).

Math (per batch row b):
    s2[b] = sum_d U2[d] * z[b, d]
    s3[b] = sum_d U3[d] * z[b, d]
    out[b, o] = (s2[b] * s3[b]) * sum_d (U1[d] * z[b, d]) * W[o, d] + bias[o]

Sharding: data-parallel over batch B=8192 across 8 NeuronCores
(B_loc = 1024 rows per core); W / U1 / U2 / U3 / bias replicated.

All-bf16 pipeline (measured rel-err 0.29% vs the 2e-2 gate), main matmul
output-natural (psum [b, o]) so there are no output transposes. The
schedule is built for overlap: everything is emitted per batch-tile-PAIR
so the main matmul stream can start ~12us in and fill the z-DMA gaps
(v3's phase-serial variant left 37us of PE idle; a variant with s2/s3 on
DVE ran the whole PE at 2.0 GHz -- P0 power state -- so s2/s3 stays on
PE and DVE only does the U1 fold and psum evictions).

  A. z bf16 row-tiles via SWDGE; PE transposes (bf16 = 1 cyc/row) in
     4-chunk groups through PSUM; ACT copies into resident
     ztbig = z.T [128 d, k(32) * 1024 b], raw.
  B. Per PAIR of batch tiles: s2/s3 on PE: psum[64, 256] += u23pad.T @
     ztRAW over 32 k (U2 -> stationary col 0, U3 -> col 32 so evictions
     read 32-aligned psum partitions).
  D. U1 folds into zt in place per (k, pair) on DVE (u1 on partitions)
     -- the only gate for that pair's main matmuls.
  C. After the last pair: c = s2*s3 (DVE) -> 8 one-column micro-matmuls
     -> ccol [128 b, 8 bt] (c becomes a per-partition scalar at
     eviction); bias broadcast via ones-outer-product matmuls -> biasb.
  E. Per o-chunk (8 x 512): wt slab [128 d, 32 k, 512 o] via SWDGE (the
     first slab is split in two k-halves and hoisted between z0 and z1
     so oc0 can start as soon as one batch pair is ready); per bt:
     psum[128 b, 512 o] += zt[k, bt] (stationary) @ wt[k, oc] (moving);
     evict with ONE DVE op: (psum * ccol) + biasb; batched out DMA per
     oc, split in half for the last chunk to shorten the drain tail.

Host prep is dtype/layout only: bf16 casts, W.T contiguous, u1/u23
pre-tiled to [128, 32(,2)] (partition-contiguous one-shot loads), and an
identity matrix (replaces on-device iota codegen that delayed the z DMA
queue).
"""

import os
import sys

import numpy as np

if "/opt/trn_rl_repo" not in sys.path:
    sys.path.insert(0, "/opt/trn_rl_repo")

import concourse.bass as bass
from concourse import bacc
import concourse.mybir as mybir
import concourse.tile as tile

P = 128
D = 4096
O = 4096
B = 8192
NCORES = 8
BLOC = B // NCORES          # 1024 batch rows per core
KC = D // P                 # 32 contraction chunks
BT = BLOC // P              # 8 batch tiles of 128
NP = BT // 2                # 4 batch-tile pairs
OC = O // 512               # 8 output chunks of 512
KH = KC // 2                # k-half for the hoisted first W slab
F32 = mybir.dt.float32
BF16 = mybir.dt.bfloat16
MULT = mybir.AluOpType.mult
ADD = mybir.AluOpType.add
COPY = mybir.ActivationFunctionType.Copy


def build_nc() -> bass.Bass:
    nc = bacc.Bacc(trn_type="TRN2")

    z_d = nc.dram_tensor("z", [BLOC, D], BF16, kind="ExternalInput")
    wt_d = nc.dram_tensor("wt", [D, O], BF16, kind="ExternalInput")
    u1_d = nc.dram_tensor("u1", [P, KC], F32, kind="ExternalInput")
    u23_d = nc.dram_tensor("u23", [P, KC, 2], BF16, kind="ExternalInput")
    bias_d = nc.dram_tensor("bias", [O], BF16, kind="ExternalInput")
    ident_d = nc.dram_tensor("ident", [P, P], BF16, kind="ExternalInput")
    out_d = nc.dram_tensor("out", [BLOC, O], F32, kind="ExternalOutput")

    zview = z_d[:].rearrange("(t p) d -> p t d", p=P)          # [128, 8, 4096]
    wview = wt_d[:].rearrange("(k p) o -> p k o", p=P)         # [128, 32, 4096]
    oview = out_d[:].rearrange("(t p) o -> p t o", p=P)        # [128, 8, 4096]

    with tile.TileContext(nc) as tc:
        with (
            tc.tile_pool(name="const", bufs=1) as const,
            tc.tile_pool(name="ztp", bufs=1) as ztp,
            tc.tile_pool(name="znat", bufs=2) as znatp,
            tc.tile_pool(name="wslab", bufs=2) as wslabp,
            tc.tile_pool(name="onat", bufs=2) as onatp,
            tc.tile_pool(name="pmain", bufs=5, space="PSUM") as pmain,
            tc.tile_pool(name="ptr", bufs=2, space="PSUM") as ptr,
            tc.tile_pool(name="pmisc", bufs=1, space="PSUM") as pmisc,
        ):
            # ---- constants (host-tiled, partition-contiguous loads) ----
            identity_b = const.tile([P, P], BF16)
            nc.sync.dma_start(identity_b[:], ident_d[:])
            ones1 = const.tile([1, P], BF16)
            nc.vector.memset(ones1[:], 1.0)
            onef = const.tile([1, 1], F32)
            nc.vector.memset(onef[:], 1.0)
            u1sb = const.tile([P, KC], F32)
            nc.sync.dma_start(u1sb[:], u1_d[:])
            u23sb = const.tile([P, KC, 2], BF16)
            nc.sync.dma_start(u23sb[:], u23_d[:])
            # s2/s3 psum rows must land on 32-aligned partitions: put U2 in
            # stationary column 0 and U3 in column 32 of a zero-padded lhsT.
            u23pad = const.tile([P, KC, 64], BF16)
            nc.vector.memset(u23pad[:], 0.0)
            nc.vector.tensor_copy(u23pad[:, :, 0:1], u23sb[:, :, 0:1])
            nc.vector.tensor_copy(u23pad[:, :, 32:33], u23sb[:, :, 1:2])
            biasrow = onatp.tile([1, O], BF16, name="onat")
            nc.sync.dma_start(biasrow[:], bias_d[:].rearrange("(a o) -> a o", a=1))
            biasb = const.tile([P, O], BF16)
            t2row = const.tile([1, BLOC], F32)
            t3row = const.tile([1, BLOC], F32)
            ccol = const.tile([P, BT], F32)

            # warm-up transpose: first PE instruction, one small DMA dep
            ptw = ptr.tile([P, 512], BF16, name="pt", tag="pt")
            nc.tensor.transpose(ptw[:, 0:P], identity_b[:], identity_b[:])

            # bias broadcast right away: fills the PE while z0 streams in
            for oc in range(OC):
                pb = pmisc.tile([P, 512], F32, name="pb", tag="pmisc")
                nc.tensor.matmul(
                    pb[:], ones1[:], biasrow[0:1, oc * 512 : (oc + 1) * 512],
                    start=True, stop=True,
                )
                nc.scalar.activation(biasb[:, oc * 512 : (oc + 1) * 512], pb[:], COPY)

            # zT resident: [128 d_in, k * BLOC + b]
            ztbig = ztp.tile([P, KC * BLOC], BF16)
            zt3 = ztbig[:].rearrange("p (k r) -> p k r", r=BLOC)

            ws0 = None
            # ---- phases A/B/D per batch-tile pair ----
            for pr in range(NP):
                for half in range(2):
                    bt = pr * 2 + half
                    znat = znatp.tile([P, D], BF16, name="znat")
                    nc.gpsimd.dma_start(znat[:], zview[:, bt, :])
                    if bt == 1:
                        # hoist the first W slab (two k-halves) after z1 on
                        # the SWDGE queue: oc0 starts early, pair0 never
                        # waits behind the 4 MiB W transfer
                        ws0 = wslabp.tile([P, KC, 512], BF16, name="wslab")
                        for h in range(2):
                            nc.gpsimd.dma_start(
                                ws0[:, h * KH : (h + 1) * KH, :],
                                wview[:, h * KH : (h + 1) * KH, 0:512],
                            )
                    for g in range(KC // 4):
                        pt = ptr.tile([P, 512], BF16, name="pt", tag="pt")
                        for i in range(4):
                            nc.tensor.transpose(
                                pt[:, i * P : (i + 1) * P],
                                znat[:, (g * 4 + i) * P : (g * 4 + i + 1) * P],
                                identity_b[:],
                            )
                        nc.scalar.activation(
                            zt3[:, g * 4 : g * 4 + 4, bt * P : (bt + 1) * P],
                            pt[:].rearrange("p (k r) -> p k r", r=P),
                            COPY,
                        )
                # B: s2/s3 for this pair from RAW zt
                sl = slice(pr * 256, (pr + 1) * 256)
                ps23 = pmisc.tile([64, 256], F32, name="ps23", tag="pmisc")
                for k in range(KC):
                    nc.tensor.matmul(
                        ps23[:],
                        u23pad[:, k, :],
                        zt3[:, k, sl],
                        start=(k == 0),
                        stop=(k == KC - 1),
                    )
                nc.vector.tensor_copy(t2row[0:1, sl], ps23[0:1, :])
                nc.vector.tensor_copy(t3row[0:1, sl], ps23[32:33, :])
                # D: fold U1 into this pair's zt in place
                for k in range(KC):
                    nc.vector.tensor_scalar_mul(
                        zt3[:, k, sl], zt3[:, k, sl], u1sb[:, k : k + 1]
                    )
                # C (per pair): c = s2*s3 -> ccol[:, 2pr:2pr+2] so the first
                # o-chunk's evictions never wait on the full prelude
                nc.vector.tensor_mul(t2row[0:1, sl], t2row[0:1, sl], t3row[0:1, sl])
                pcp = pmisc.tile([P, 2], F32, name="pc", tag="pmisc")
                for gi in range(2):
                    g = pr * 2 + gi
                    nc.tensor.matmul(
                        pcp[:, gi : gi + 1],
                        t2row[0:1, g * P : (g + 1) * P],
                        onef[0:1, 0:1],
                        start=True, stop=True,
                    )
                nc.vector.tensor_copy(ccol[:, pr * 2 : pr * 2 + 2], pcp[:])

            # ---- phase E: main matmul, output-natural psum [b, o] ----
            for oc in range(OC):
                if oc == 0:
                    ws = ws0
                else:
                    ws = wslabp.tile([P, KC, 512], BF16, name="wslab")
                    nc.gpsimd.dma_start(
                        ws[:], wview[:, :, oc * 512 : (oc + 1) * 512]
                    )
                onat = onatp.tile([P, BT, 512], F32, name="onat")
                for bt in range(BT):
                    pm = pmain.tile([P, 512], F32, name="pm", tag="pmain")
                    for k in range(KC):
                        nc.tensor.matmul(
                            pm[:],
                            zt3[:, k, bt * P : (bt + 1) * P],
                            ws[:, k, :],
                            start=(k == 0),
                            stop=(k == KC - 1),
                        )
                    nc.vector.scalar_tensor_tensor(
                        onat[:, bt, :],
                        pm[:],
                        ccol[:, bt : bt + 1],
                        biasb[:, oc * 512 : (oc + 1) * 512],
                        MULT,
                        ADD,
                    )
                if oc == OC - 1:
                    # split the last store so the drain tail is half as long
                    nc.gpsimd.dma_start(
                        oview[:, 0 : BT // 2, oc * 512 : (oc + 1) * 512],
                        onat[:, 0 : BT // 2, :],
                    )
                    nc.gpsimd.dma_start(
                        oview[:, BT // 2 : BT, oc * 512 : (oc + 1) * 512],
                        onat[:, BT // 2 : BT, :],
                    )
                else:
                    nc.gpsimd.dma_start(
                        oview[:, :, oc * 512 : (oc + 1) * 512], onat[:]
                    )

    nc.finalize()
    return nc


_NC_CACHE = {}


def get_nc() -> bass.Bass:
    if "nc" not in _NC_CACHE:
        _NC_CACHE["nc"] = build_nc()
    return _NC_CACHE["nc"]


def kernel(z, U1, U2, U3, W, b):
    import ml_dtypes
    from concourse.bass_utils import run_bass_kernel_spmd

    bf = ml_dtypes.bfloat16
    z = np.ascontiguousarray(np.asarray(z, dtype=np.float32)).reshape(B, D)
    zq = z.astype(bf)
    wt = np.ascontiguousarray(np.asarray(W, dtype=np.float32).T).astype(bf)
    u1t = np.ascontiguousarray(
        np.asarray(U1, dtype=np.float32).reshape(KC, P).T
    )
    u23 = np.stack(
        [np.asarray(U2, dtype=np.float32), np.asarray(U3, dtype=np.float32)], 1
    )
    u23t = np.ascontiguousarray(
        u23.reshape(KC, P, 2).transpose(1, 0, 2)
    ).astype(bf)
    bias = np.asarray(b, dtype=np.float32).astype(bf)
    ident = np.eye(P, dtype=bf)

    nc = get_nc()
    in_maps = [
        {
            "z": zq[c * BLOC : (c + 1) * BLOC],
            "wt": wt,
            "u1": u1t,
            "u23": u23t,
            "bias": bias,
            "ident": ident,
        }
        for c in range(NCORES)
    ]
    res = run_bass_kernel_spmd(
        nc,
        in_maps,
        core_ids=list(range(NCORES)),
        trace=bool(int(os.environ.get("KERNEL_TRACE", "0"))),
    )
    if res.exec_time_ns is not None:
        print(f"HW exec time: {res.exec_time_ns} ns", file=sys.stderr)
    kernel.last_results = res
    return np.concatenate([res.results[c]["out"] for c in range(NCORES)], axis=0)


# revision 21
# speedup vs baseline: 1.3141x; 1.0075x over previous
"""Trainium2 Bass kernel for nn_CP_L3_sparse_outer (v5, bf16).

Math (per batch row b):
    s2[b] = sum_d U2[d] * z[b, d]
    s3[b] = sum_d U3[d] * z[b, d]
    out[b, o] = (s2[b] * s3[b]) * sum_d (U1[d] * z[b, d]) * W[o, d] + bias[o]

Sharding: data-parallel over batch B=8192 across 8 NeuronCores
(B_loc = 1024 rows per core); W / U1 / U2 / U3 / bias replicated.

All-bf16 pipeline (measured rel-err 0.29% vs the 2e-2 gate), main matmul
output-natural (psum [b, o]) so there are no output transposes. The
schedule is built for overlap: everything is emitted per batch-tile-PAIR
so the main matmul stream can start ~12us in and fill the z-DMA gaps
(v3's phase-serial variant left 37us of PE idle; a variant with s2/s3 on
DVE ran the whole PE at 2.0 GHz -- P0 power state -- so s2/s3 stays on
PE and DVE only does the U1 fold and psum evictions).

  A. z bf16 row-tiles via SWDGE; PE transposes (bf16 = 1 cyc/row) in
     4-chunk groups through PSUM; ACT copies into resident
     ztbig = z.T [128 d, k(32) * 1024 b], raw.
  B. Per PAIR of batch tiles: s2/s3 on PE: psum[64, 256] += u23pad.T @
     ztRAW over 32 k (U2 -> stationary col 0, U3 -> col 32 so evictions
     read 32-aligned psum partitions).
  D. U1 folds into zt in place per (k, pair) on DVE (u1 on partitions)
     -- the only gate for that pair's main matmuls.
  C. After the last pair: c = s2*s3 (DVE) -> 8 one-column micro-matmuls
     -> ccol [128 b, 8 bt] (c becomes a per-partition scalar at
     eviction); bias broadcast via ones-outer-product matmuls -> biasb.
  E. Per o-chunk (8 x 512): wt slab [128 d, 32 k, 512 o] via SWDGE (the
     first slab is split in two k-halves and hoisted between z0 and z1
     so oc0 can start as soon as one batch pair is ready); per bt:
     psum[128 b, 512 o] += zt[k, bt] (stationary) @ wt[k, oc] (moving);
     evict with ONE DVE op: (psum * ccol) + biasb; batched out DMA per
     oc, split in half for the last chunk to shorten the drain tail.

Host prep is dtype/layout only: bf16 casts, W.T contiguous, u1/u23
pre-tiled to [128, 32(,2)] (partition-contiguous one-shot loads), and an
identity matrix (replaces on-device iota codegen that delayed the z DMA
queue).
"""

import os
import sys

import numpy as np

if "/opt/trn_rl_repo" not in sys.path:
    sys.path.insert(0, "/opt/trn_rl_repo")

import concourse.bass as bass
from concourse import bacc
import concourse.mybir as mybir
import concourse.tile as tile

P = 128
D = 4096
O = 4096
B = 8192
NCORES = 8
BLOC = B // NCORES          # 1024 batch rows per core
KC = D // P                 # 32 contraction chunks
BT = BLOC // P              # 8 batch tiles of 128
NP = BT // 2                # 4 batch-tile pairs
OC = O // 512               # 8 output chunks of 512
KH = KC // 2                # k-half for the hoisted first W slab
F32 = mybir.dt.float32
BF16 = mybir.dt.bfloat16
MULT = mybir.AluOpType.mult
ADD = mybir.AluOpType.add
COPY = mybir.ActivationFunctionType.Copy


def build_nc() -> bass.Bass:
    nc = bacc.Bacc(trn_type="TRN2")

    z_d = nc.dram_tensor("z", [BLOC, D], BF16, kind="ExternalInput")
    wt_d = nc.dram_tensor("wt", [D, O], BF16, kind="ExternalInput")
    u1_d = nc.dram_tensor("u1", [P, KC], F32, kind="ExternalInput")
    u23_d = nc.dram_tensor("u23", [P, KC, 2], BF16, kind="ExternalInput")
    bias_d = nc.dram_tensor("bias", [O], BF16, kind="ExternalInput")
    ident_d = nc.dram_tensor("ident", [P, P], BF16, kind="ExternalInput")
    out_d = nc.dram_tensor("out", [BLOC, O], F32, kind="ExternalOutput")

    zview = z_d[:].rearrange("(t p) d -> p t d", p=P)          # [128, 8, 4096]
    wview = wt_d[:].rearrange("(k p) o -> p k o", p=P)         # [128, 32, 4096]
    oview = out_d[:].rearrange("(t p) o -> p t o", p=P)        # [128, 8, 4096]

    with tile.TileContext(nc) as tc:
        with (
            tc.tile_pool(name="const", bufs=1) as const,
            tc.tile_pool(name="ztp", bufs=1) as ztp,
            tc.tile_pool(name="znat", bufs=2) as znatp,
            tc.tile_pool(name="wslab", bufs=2) as wslabp,
            tc.tile_pool(name="onat", bufs=2) as onatp,
            tc.tile_pool(name="pmain", bufs=4, space="PSUM") as pmain,
            tc.tile_pool(name="ptr", bufs=3, space="PSUM") as ptr,
            tc.tile_pool(name="pmisc", bufs=1, space="PSUM") as pmisc,
        ):
            # ---- constants (host-tiled, partition-contiguous loads) ----
            identity_b = const.tile([P, P], BF16)
            nc.sync.dma_start(identity_b[:], ident_d[:])
            ones1 = const.tile([1, P], BF16)
            nc.vector.memset(ones1[:], 1.0)
            onef = const.tile([1, 1], F32)
            nc.vector.memset(onef[:], 1.0)
            # biasrow loads second on the sync queue: the bias broadcast
            # matmuls are the first real PE work after the warm-up
            biasrow = onatp.tile([1, O], BF16, name="onat")
            nc.sync.dma_start(biasrow[:], bias_d[:].rearrange("(a o) -> a o", a=1))
            u1sb = const.tile([P, KC], F32)
            nc.sync.dma_start(u1sb[:], u1_d[:])
            u23sb = const.tile([P, KC, 2], BF16)
            nc.sync.dma_start(u23sb[:], u23_d[:])
            # s2/s3 psum rows must land on 32-aligned partitions: put U2 in
            # stationary column 0 and U3 in column 32 of a zero-padded lhsT.
            u23pad = const.tile([P, KC, 64], BF16)
            nc.vector.memset(u23pad[:], 0.0)
            nc.vector.tensor_copy(u23pad[:, :, 0:1], u23sb[:, :, 0:1])
            nc.vector.tensor_copy(u23pad[:, :, 32:33], u23sb[:, :, 1:2])
            biasb = const.tile([P, O], BF16)
            t2row = const.tile([1, BLOC], F32)
            t3row = const.tile([1, BLOC], F32)
            ccol = const.tile([P, BT], F32)

            # warm-up transpose: first PE instruction, one small DMA dep
            ptw = ptr.tile([P, 512], BF16, name="pt", tag="pt")
            nc.tensor.transpose(ptw[:, 0:P], identity_b[:], identity_b[:])

            # bias broadcast right away: fills the PE while z0 streams in
            for oc in range(OC):
                pb = pmisc.tile([P, 512], F32, name="pb", tag="pmisc")
                nc.tensor.matmul(
                    pb[:], ones1[:], biasrow[0:1, oc * 512 : (oc + 1) * 512],
                    start=True, stop=True,
                )
                nc.scalar.activation(biasb[:, oc * 512 : (oc + 1) * 512], pb[:], COPY)

            # zT resident: [128 d_in, k * BLOC + b]
            ztbig = ztp.tile([P, KC * BLOC], BF16)
            zt3 = ztbig[:].rearrange("p (k r) -> p k r", r=BLOC)

            ws0 = None
            # ---- phases A/B/D per batch-tile pair ----
            for pr in range(NP):
                for half in range(2):
                    bt = pr * 2 + half
                    znat = znatp.tile([P, D], BF16, name="znat")
                    nc.gpsimd.dma_start(znat[:], zview[:, bt, :])
                    if bt == 1:
                        # hoist the first W slab (two k-halves) after z1 on
                        # the SWDGE queue: oc0 starts early, pair0 never
                        # waits behind the 4 MiB W transfer
                        ws0 = wslabp.tile([P, KC, 512], BF16, name="wslab")
                        for h in range(2):
                            nc.gpsimd.dma_start(
                                ws0[:, h * KH : (h + 1) * KH, :],
                                wview[:, h * KH : (h + 1) * KH, 0:512],
                            )
                    for g in range(KC // 4):
                        pt = ptr.tile([P, 512], BF16, name="pt", tag="pt")
                        for i in range(4):
                            nc.tensor.transpose(
                                pt[:, i * P : (i + 1) * P],
                                znat[:, (g * 4 + i) * P : (g * 4 + i + 1) * P],
                                identity_b[:],
                            )
                        nc.scalar.activation(
                            zt3[:, g * 4 : g * 4 + 4, bt * P : (bt + 1) * P],
                            pt[:].rearrange("p (k r) -> p k r", r=P),
                            COPY,
                        )
                # B: s2/s3 for this pair from RAW zt
                sl = slice(pr * 256, (pr + 1) * 256)
                ps23 = pmisc.tile([64, 256], F32, name="ps23", tag="pmisc")
                for k in range(KC):
                    nc.tensor.matmul(
                        ps23[:],
                        u23pad[:, k, :],
                        zt3[:, k, sl],
                        start=(k == 0),
                        stop=(k == KC - 1),
                    )
                nc.vector.tensor_copy(t2row[0:1, sl], ps23[0:1, :])
                nc.vector.tensor_copy(t3row[0:1, sl], ps23[32:33, :])
                # D: fold U1 into this pair's zt in place
                for k in range(KC):
                    nc.vector.tensor_scalar_mul(
                        zt3[:, k, sl], zt3[:, k, sl], u1sb[:, k : k + 1]
                    )
                # C (per pair): c = s2*s3 -> ccol[:, 2pr:2pr+2] so the first
                # o-chunk's evictions never wait on the full prelude
                nc.vector.tensor_mul(t2row[0:1, sl], t2row[0:1, sl], t3row[0:1, sl])
                pcp = pmisc.tile([P, 2], F32, name="pc", tag="pmisc")
                for gi in range(2):
                    g = pr * 2 + gi
                    nc.tensor.matmul(
                        pcp[:, gi : gi + 1],
                        t2row[0:1, g * P : (g + 1) * P],
                        onef[0:1, 0:1],
                        start=True, stop=True,
                    )
                nc.vector.tensor_copy(ccol[:, pr * 2 : pr * 2 + 2], pcp[:])

            # ---- phase E: main matmul, output-natural psum [b, o] ----
            for oc in range(OC):
                if oc == 0:
                    ws = ws0
                else:
                    ws = wslabp.tile([P, KC, 512], BF16, name="wslab")
                    nc.gpsimd.dma_start(
                        ws[:], wview[:, :, oc * 512 : (oc + 1) * 512]
                    )
                onat = onatp.tile([P, BT, 512], F32, name="onat")
                for bt in range(BT):
                    pm = pmain.tile([P, 512], F32, name="pm", tag="pmain")
                    for k in range(KC):
                        nc.tensor.matmul(
                            pm[:],
                            zt3[:, k, bt * P : (bt + 1) * P],
                            ws[:, k, :],
                            start=(k == 0),
                            stop=(k == KC - 1),
                        )
                    nc.vector.scalar_tensor_tensor(
                        onat[:, bt, :],
                        pm[:],
                        ccol[:, bt : bt + 1],
                        biasb[:, oc * 512 : (oc + 1) * 512],
                        MULT,
                        ADD,
                    )
                if oc == OC - 1:
                    # split the last store so the drain tail shrinks
                    for q in range(4):
                        nc.gpsimd.dma_start(
                            oview[:, 2 * q : 2 * q + 2, oc * 512 : (oc + 1) * 512],
                            onat[:, 2 * q : 2 * q + 2, :],
                        )
                else:
                    nc.gpsimd.dma_start(
                        oview[:, :, oc * 512 : (oc + 1) * 512], onat[:]
                    )

    nc.finalize()
    return nc


_NC_CACHE = {}


def get_nc() -> bass.Bass:
    if "nc" not in _NC_CACHE:
        _NC_CACHE["nc"] = build_nc()
    return _NC_CACHE["nc"]


def kernel(z, U1, U2, U3, W, b):
    import ml_dtypes
    from concourse.bass_utils import run_bass_kernel_spmd

    bf = ml_dtypes.bfloat16
    z = np.ascontiguousarray(np.asarray(z, dtype=np.float32)).reshape(B, D)
    zq = z.astype(bf)
    wt = np.ascontiguousarray(np.asarray(W, dtype=np.float32).T).astype(bf)
    u1t = np.ascontiguousarray(
        np.asarray(U1, dtype=np.float32).reshape(KC, P).T
    )
    u23 = np.stack(
        [np.asarray(U2, dtype=np.float32), np.asarray(U3, dtype=np.float32)], 1
    )
    u23t = np.ascontiguousarray(
        u23.reshape(KC, P, 2).transpose(1, 0, 2)
    ).astype(bf)
    bias = np.asarray(b, dtype=np.float32).astype(bf)
    ident = np.eye(P, dtype=bf)

    nc = get_nc()
    in_maps = [
        {
            "z": zq[c * BLOC : (c + 1) * BLOC],
            "wt": wt,
            "u1": u1t,
            "u23": u23t,
            "bias": bias,
            "ident": ident,
        }
        for c in range(NCORES)
    ]
    res = run_bass_kernel_spmd(
        nc,
        in_maps,
        core_ids=list(range(NCORES)),
        trace=bool(int(os.environ.get("KERNEL_TRACE", "0"))),
    )
    if res.exec_time_ns is not None:
        print(f"HW exec time: {res.exec_time_ns} ns", file=sys.stderr)
    kernel.last_results = res
    return np.concatenate([res.results[c]["out"] for c in range(NCORES)], axis=0)


# revision 22
# speedup vs baseline: 1.3488x; 1.0264x over previous
"""Trainium2 Bass kernel for nn_CP_L3_sparse_outer (v8, bf16).

Math (per batch row b):
    s2[b] = sum_d U2[d] * z[b, d]
    s3[b] = sum_d U3[d] * z[b, d]
    out[b, o] = (s2[b] * s3[b]) * sum_d (U1[d] * z[b, d]) * W[o, d] + bias[o]

Sharding: data-parallel over batch B=8192 across 8 NeuronCores
(B_loc = 1024 rows per core); W / U1 / U2 / U3 / bias replicated.

All-bf16 pipeline (measured rel-err 0.29% vs the 2e-2 gate), main matmul
output-natural (psum [b, o]): no output transposes, and z arrives
PRE-TRANSPOSED from the host (pure layout prep, same as W.T), so there
are no input transposes either -- the tensor engine runs only the s2/s3
reductions and the 2048-matmul main stream, which issues back-to-back at
the 216 ns N=512 roofline.

  A. zT bf16 [128 d, k(32), 1024 b] streams straight into resident ztbig
     via SWDGE, one DMA per batch-tile PAIR (256 cols) for pipelining.
  B. Per pair: s2/s3 on PE from raw zT: psum[64, 256] += u23pad.T @ zt
     over 32 k (U2 -> stationary col 0, U3 -> col 32: psum partitions
     must be 32-aligned for the evicting copies).
  D. U1 folds into zt in place per (k, pair) on DVE (u1 on partitions)
     -- the only gate for that pair's main matmuls.
  C. Per pair: c = s2*s3 (DVE) -> 2 one-column micro-matmuls -> ccol
     [128 b, 8 bt] (c becomes a per-partition scalar at eviction).
  E. Per o-chunk (8 x 512): wt slab [128 d, 32 k, 512 o] via SWDGE (the
     first slab is split in two k-halves and hoisted behind pair0's zT
     load); per bt: psum[128 b, 512 o] += zt[k, bt] (stationary) @
     wt[k, oc] (moving); evict with ONE DVE op: (psum * ccol) + biasb;
     batched out DMA per oc, quartered for the last chunk to shorten the
     drain tail.

bias[o] sits on the free dim at eviction, so it is broadcast across
partitions once via ones-outer-product matmuls (the first PE work, which
also serves as warm-up while zT streams in). Host prep is dtype/layout
only: bf16 casts, z.T / W.T contiguous, u1/u23 pre-tiled to
[128, 32(,2)] so every one-shot load is partition-contiguous.

History (HW-measured): f32r staged baseline 660,683 ns; v2 flipped-bf16
545,755; v6 overlap fixes 518,382; v7 psum/ordering 514,509. A variant
with s2/s3 on DVE accumulators ran the PE at 2.0 GHz (P0 power state,
259 ns/matmul) -- keep s2/s3 on the tensor engine.
"""

import os
import sys

import numpy as np

if "/opt/trn_rl_repo" not in sys.path:
    sys.path.insert(0, "/opt/trn_rl_repo")

import concourse.bass as bass
from concourse import bacc
import concourse.mybir as mybir
import concourse.tile as tile

P = 128
D = 4096
O = 4096
B = 8192
NCORES = 8
BLOC = B // NCORES          # 1024 batch rows per core
KC = D // P                 # 32 contraction chunks
BT = BLOC // P              # 8 batch tiles of 128
NP = BT // 2                # 4 batch-tile pairs
OC = O // 512               # 8 output chunks of 512
KH = KC // 2                # k-half for the hoisted first W slab
F32 = mybir.dt.float32
BF16 = mybir.dt.bfloat16
MULT = mybir.AluOpType.mult
ADD = mybir.AluOpType.add
COPY = mybir.ActivationFunctionType.Copy


def build_nc() -> bass.Bass:
    nc = bacc.Bacc(trn_type="TRN2")

    zt_d = nc.dram_tensor("zt", [D, BLOC], BF16, kind="ExternalInput")
    wt_d = nc.dram_tensor("wt", [D, O], BF16, kind="ExternalInput")
    u1_d = nc.dram_tensor("u1", [P, KC], F32, kind="ExternalInput")
    u23_d = nc.dram_tensor("u23", [P, KC, 2], BF16, kind="ExternalInput")
    bias_d = nc.dram_tensor("bias", [O], BF16, kind="ExternalInput")
    out_d = nc.dram_tensor("out", [BLOC, O], F32, kind="ExternalOutput")

    ztv = zt_d[:].rearrange("(k p) b -> p k b", p=P)           # [128, 32, 1024]
    wview = wt_d[:].rearrange("(k p) o -> p k o", p=P)         # [128, 32, 4096]
    oview = out_d[:].rearrange("(t p) o -> p t o", p=P)        # [128, 8, 4096]

    with tile.TileContext(nc) as tc:
        with (
            tc.tile_pool(name="const", bufs=1) as const,
            tc.tile_pool(name="ztp", bufs=1) as ztp,
            tc.tile_pool(name="wslab", bufs=2) as wslabp,
            tc.tile_pool(name="onat", bufs=2) as onatp,
            tc.tile_pool(name="pmain", bufs=6, space="PSUM") as pmain,
            tc.tile_pool(name="pmisc", bufs=2, space="PSUM") as pmisc,
        ):
            # ---- constants (host-tiled, partition-contiguous loads) ----
            ones1 = const.tile([1, P], BF16)
            nc.vector.memset(ones1[:], 1.0)
            onef = const.tile([1, 1], F32)
            nc.vector.memset(onef[:], 1.0)
            # biasrow first on the sync queue: the bias broadcast matmuls
            # are the PE warm-up while zT streams in
            biasrow = onatp.tile([1, O], BF16, name="onat")
            nc.sync.dma_start(biasrow[:], bias_d[:].rearrange("(a o) -> a o", a=1))
            u1sb = const.tile([P, KC], F32)
            nc.sync.dma_start(u1sb[:], u1_d[:])
            u23sb = const.tile([P, KC, 2], BF16)
            nc.sync.dma_start(u23sb[:], u23_d[:])
            # s2/s3 psum rows must land on 32-aligned partitions: put U2 in
            # stationary column 0 and U3 in column 32 of a zero-padded lhsT.
            u23pad = const.tile([P, KC, 64], BF16)
            nc.vector.memset(u23pad[:], 0.0)
            nc.vector.tensor_copy(u23pad[:, :, 0:1], u23sb[:, :, 0:1])
            nc.vector.tensor_copy(u23pad[:, :, 32:33], u23sb[:, :, 1:2])
            biasb = const.tile([P, O], BF16)
            t2row = const.tile([1, BLOC], F32)
            t3row = const.tile([1, BLOC], F32)
            ccol = const.tile([P, BT], F32)

            # bias broadcast: first PE instructions (also HAM warm-up)
            for oc in range(OC):
                pb = pmisc.tile([P, 512], F32, name="pb", tag="pmisc")
                nc.tensor.matmul(
                    pb[:], ones1[:], biasrow[0:1, oc * 512 : (oc + 1) * 512],
                    start=True, stop=True,
                )
                nc.scalar.activation(biasb[:, oc * 512 : (oc + 1) * 512], pb[:], COPY)

            # zT resident: [128 d_in, k * BLOC + b]
            ztbig = ztp.tile([P, KC * BLOC], BF16)
            zt3 = ztbig[:].rearrange("p (k r) -> p k r", r=BLOC)

            ws0 = None
            # ---- phases A/B/D/C per batch-tile pair ----
            for pr in range(NP):
                sl = slice(pr * 256, (pr + 1) * 256)
                # A: zT pair streams straight into ztbig (no transposes)
                nc.gpsimd.dma_start(zt3[:, :, sl], ztv[:, :, sl])
                if pr == 0:
                    # hoist the first W slab (two k-halves) behind pair0
                    ws0 = wslabp.tile([P, KC, 512], BF16, name="wslab")
                    for h in range(2):
                        nc.gpsimd.dma_start(
                            ws0[:, h * KH : (h + 1) * KH, :],
                            wview[:, h * KH : (h + 1) * KH, 0:512],
                        )
                # B: s2/s3 for this pair from RAW zt
                ps23 = pmisc.tile([64, 256], F32, name="ps23", tag="pmisc")
                for k in range(KC):
                    nc.tensor.matmul(
                        ps23[:],
                        u23pad[:, k, :],
                        zt3[:, k, sl],
                        start=(k == 0),
                        stop=(k == KC - 1),
                    )
                nc.vector.tensor_copy(t2row[0:1, sl], ps23[0:1, :])
                nc.vector.tensor_copy(t3row[0:1, sl], ps23[32:33, :])
                # D: fold U1 into this pair's zt in place
                for k in range(KC):
                    nc.vector.tensor_scalar_mul(
                        zt3[:, k, sl], zt3[:, k, sl], u1sb[:, k : k + 1]
                    )
                # C (per pair): c = s2*s3 -> ccol[:, 2pr:2pr+2] so the first
                # o-chunk's evictions never wait on the full prelude
                nc.vector.tensor_mul(t2row[0:1, sl], t2row[0:1, sl], t3row[0:1, sl])
                pcp = pmisc.tile([P, 2], F32, name="pc", tag="pmisc")
                for gi in range(2):
                    g = pr * 2 + gi
                    nc.tensor.matmul(
                        pcp[:, gi : gi + 1],
                        t2row[0:1, g * P : (g + 1) * P],
                        onef[0:1, 0:1],
                        start=True, stop=True,
                    )
                nc.vector.tensor_copy(ccol[:, pr * 2 : pr * 2 + 2], pcp[:])

            # ---- phase E: main matmul, output-natural psum [b, o] ----
            for oc in range(OC):
                if oc == 0:
                    ws = ws0
                else:
                    ws = wslabp.tile([P, KC, 512], BF16, name="wslab")
                    nc.gpsimd.dma_start(
                        ws[:], wview[:, :, oc * 512 : (oc + 1) * 512]
                    )
                onat = onatp.tile([P, BT, 512], F32, name="onat")
                for bt in range(BT):
                    pm = pmain.tile([P, 512], F32, name="pm", tag="pmain")
                    for k in range(KC):
                        nc.tensor.matmul(
                            pm[:],
                            zt3[:, k, bt * P : (bt + 1) * P],
                            ws[:, k, :],
                            start=(k == 0),
                            stop=(k == KC - 1),
                        )
                    nc.vector.scalar_tensor_tensor(
                        onat[:, bt, :],
                        pm[:],
                        ccol[:, bt : bt + 1],
                        biasb[:, oc * 512 : (oc + 1) * 512],
                        MULT,
                        ADD,
                    )
                if oc == OC - 1:
                    # split the last store so the drain tail shrinks
                    for q in range(4):
                        nc.gpsimd.dma_start(
                            oview[:, 2 * q : 2 * q + 2, oc * 512 : (oc + 1) * 512],
                            onat[:, 2 * q : 2 * q + 2, :],
                        )
                else:
                    nc.gpsimd.dma_start(
                        oview[:, :, oc * 512 : (oc + 1) * 512], onat[:]
                    )

    nc.finalize()
    return nc


_NC_CACHE = {}


def get_nc() -> bass.Bass:
    if "nc" not in _NC_CACHE:
        _NC_CACHE["nc"] = build_nc()
    return _NC_CACHE["nc"]


def kernel(z, U1, U2, U3, W, b):
    import ml_dtypes
    from concourse.bass_utils import run_bass_kernel_spmd

    bf = ml_dtypes.bfloat16
    z = np.ascontiguousarray(np.asarray(z, dtype=np.float32)).reshape(B, D)
    zq = z.astype(bf)
    wt = np.ascontiguousarray(np.asarray(W, dtype=np.float32).T).astype(bf)
    u1t = np.ascontiguousarray(
        np.asarray(U1, dtype=np.float32).reshape(KC, P).T
    )
    u23 = np.stack(
        [np.asarray(U2, dtype=np.float32), np.asarray(U3, dtype=np.float32)], 1
    )
    u23t = np.ascontiguousarray(
        u23.reshape(KC, P, 2).transpose(1, 0, 2)
    ).astype(bf)
    bias = np.asarray(b, dtype=np.float32).astype(bf)

    nc = get_nc()
    in_maps = [
        {
            "zt": np.ascontiguousarray(zq[c * BLOC : (c + 1) * BLOC].T),
            "wt": wt,
            "u1": u1t,
            "u23": u23t,
            "bias": bias,
        }
        for c in range(NCORES)
    ]
    res = run_bass_kernel_spmd(
        nc,
        in_maps,
        core_ids=list(range(NCORES)),
        trace=bool(int(os.environ.get("KERNEL_TRACE", "0"))),
    )
    if res.exec_time_ns is not None:
        print(f"HW exec time: {res.exec_time_ns} ns", file=sys.stderr)
    kernel.last_results = res
    return np.concatenate([res.results[c]["out"] for c in range(NCORES)], axis=0)
